# revision 72
# baseline (speedup 1.0000x reference)
"""Enformer multi-head attention with central-mask relative position bias.

Trainium2 Bass/Tile kernel, sharded over 8 NeuronCores.

Problem (fp32): x [2, 1024, 768]; H=8 heads, dqk=dv=64, n_pos=64.
  q,k,v = x @ {Wq,Wk,Wv}.T ; basis[i,j,:] = f(j-i)  (Toeplitz!)
  qr = (q @ w_pos) . basis ; uk = u.k ; vr = (v_bias.w_pos) . basis
  scores = (q.k + qr + uk + vr)/8 ; out = softmax(scores) @ v @ Wo.T + bo

Sharding: core c owns heads {2a, 2a+1} (a = c%4) of batch c//4, so
each core touches only one batch's x (1.5 MB) and the v projection
streams 128 output columns (2 heads at once).  The output is
row-sharded so dest core c owns i-tile t=c of BOTH batches: every src
has exactly one [128 i, 128 hd] av pair tile per dest and the reshard
is a uniform 8-way AllToAll (256 KB, post-loop; firing a collective
mid-loop starves the pipeline because it monopolizes the DMA engines).

Pipeline (16 units u: h=u%2, t=u//2).  Stages are staggered so a DMA
issue never head-of-line blocks a dependent queue behind a live wait
(in particular, exp is always first on the ACT queue each iteration):
  StMM(v)  @ iter v-3: strip matmuls + PSUM->SBUF copies (DVE)
  StOUT(v) @ iter v-3: strip SBUF->DRAM bounce          (scalar queue)
  StSKEW(v)@ iter v-2: skewed DRAM->SBUF read, 1 DMA     (scalar queue)
The v projection is spread over the first two loop iterations so the
first scores matmul starts as soon as head 0's projections land.
  Sc(u)    @ iter u:   scores q.k (2 matmuls); qr then added into the
             scores PSUM by DVE scalar_tensor_tensor (no PE identity
             matmuls)
  BsH(u)   @ iter u+1: exp [128,1024] (ACT) + p XBAR transpose (sync)
  Cav(u)   @ iter u+2: attn@v (65 cols: col 64 = den via ones column
             in v) + recip + scale (DVE)
  Cout(u)  @ iter u+2, h=1 only: av pair DMA to the AllToAll staging
             buffer (scalar queue)
All XBAR transposes stay on the sync queue: two concurrent
InstDmaTransposeAnt on different queues corrupt each other (observed
on HW: only row 0 of each 16-row xbar tile lands).

Relative-position trick: basis[i,j,:] = B[j-i+1023, :] depends only on
the diagonal, so qr[i,j] + vr[i,j] = T'[i, j-i+1023] with T' = (qw+vw)
@ B.T.  T' is computed in 128-row strips [128, 1152] (the diagonal
window of one i-tile), bounced through DRAM, and read back with a
skewed access pattern (partition stride 1151 elements) that turns
diagonals into rows.  uk[j] is folded into the scores matmul as a 65th
contraction row (q row 64 = ones, k row 64 = uk); vw is folded into the
qw matmul the same way.
"""

import sys

sys.path.insert(0, "/opt/trn_rl_repo")

import numpy as np

import concourse.bass as bass
import concourse.mybir as mybir
import concourse.tile as tile
from concourse import bacc
from concourse.bass_utils import run_bass_kernel_spmd

N_CORES = 8
B, L, DM = 2, 1024, 768
H, DQK, DV, POS = 8, 64, 64, 64
ROWS = B * L             # 2048
SHARD = ROWS // N_CORES  # 256
NT = L // 128            # 8 i-tiles per batch
NU = 16                  # units per core: 2 heads x 8 i-tiles
STRIP_W = 1152           # 3 matmul chunks: 512 + 512 + 128 (window is 1151)
F32 = mybir.dt.float32
F16 = mybir.dt.float16


def _basis_bt() -> np.ndarray:
    """B.T [64, 2048] fp16: basis value for signed distance d = r - 1023.

    Values are in {-1, 0, 1}: exact in fp16.  Col 2047 is padding.
    """
    half = POS // 2
    d = np.arange(-(L - 1), L, dtype=np.int64)  # [2047]
    log_v = np.log(np.float32((L + 1) / 2.0)).astype(np.float32)
    pow_rate = np.exp(log_v / np.float32(half)).astype(np.float32)
    widths = (pow_rate ** np.arange(1, half + 1, dtype=np.float32)).astype(np.float32)
    unsigned = np.abs(d)[:, None].astype(np.float32) <= widths[None, :]
    signed = np.sign(d)[:, None] * unsigned
    bmat = np.concatenate(
        [unsigned.astype(np.float32), signed.astype(np.float32)], axis=1
    )  # [2047, 64]
    bt = np.zeros((POS, 2 * L), np.float32)
    bt[:, : 2 * L - 1] = bmat.T
    return bt.astype(np.float16)


def _build_program():
    nc = bacc.Bacc("TRN2", target_bir_lowering=False, debug=False, num_devices=N_CORES)

    xT = nc.dram_tensor("xT", [DM, L], F16, kind="ExternalInput")
    wqk2 = nc.dram_tensor("wqk2", [DM, 256], F16, kind="ExternalInput")
    wv2 = nc.dram_tensor("wv2", [DM, 128], F16, kind="ExternalInput")
    # pre-transposed on host: [65, 2, 64] / [64, 2, 65] load contiguously
    wposa2 = nc.dram_tensor("wposa2", [DQK + 1, 2, POS], F16, kind="ExternalInput")
    uaug2 = nc.dram_tensor("uaug2", [DQK, 2, DQK + 1], F16, kind="ExternalInput")
    wo = nc.dram_tensor("wo", [H * DV, DM], F16, kind="ExternalInput")
    bo = nc.dram_tensor("bo", [1, DM], F16, kind="ExternalInput")
    out = nc.dram_tensor("out_shard", [SHARD, DM], F32, kind="ExternalOutput")

    bt_const = nc.inline_tensor(_basis_bt(), name="bt_const")

    with tile.TileContext(nc) as tc:
        _emit(nc, tc, xT, wqk2, wv2, wposa2, uaug2, wo, bo, bt_const, out)
    nc.compile()
    return nc


def _emit(nc, tc, xT, wqk2, wv2, wposa2, uaug2, wo, bo, bt_const, out):
    import contextlib

    ctx = contextlib.ExitStack()
    with ctx:
        consts = ctx.enter_context(tc.tile_pool(name="consts", bufs=1))
        perb = ctx.enter_context(tc.tile_pool(name="perb", bufs=1))
        work = ctx.enter_context(tc.tile_pool(name="work", bufs=6))
        pwork = ctx.enter_context(tc.tile_pool(name="pwork", bufs=5))
        ps_st = ctx.enter_context(tc.tile_pool(name="ps_st", bufs=2, space="PSUM"))
        ps_av = ctx.enter_context(tc.tile_pool(name="ps_av", bufs=2, space="PSUM"))
        ps_sc = ctx.enter_context(tc.tile_pool(name="ps_sc", bufs=2, space="PSUM"))
        dram = ctx.enter_context(tc.tile_pool(name="dram", bufs=6, space="DRAM"))

        scale = 1.0 / np.sqrt(DQK)

        # ---- constants: x on sync, weights on scalar (parallel queues) ----
        xT_sb = consts.tile([128, 6, L], F16)
        for cc in range(2):
            nc.sync.dma_start(
                out=xT_sb[:, :, cc * 512 : (cc + 1) * 512],
                in_=xT[:].rearrange("(c p) i -> p c i", p=128)[
                    :, :, cc * 512 : (cc + 1) * 512
                ],
            )
        wqk_sb = consts.tile([128, 6, 256], F16)
        nc.scalar.dma_start(out=wqk_sb, in_=wqk2[:].rearrange("(c p) m -> p c m", p=128))
        wposa_sb = consts.tile([DQK + 1, 2, POS], F16)
        nc.scalar.dma_start(out=wposa_sb, in_=wposa2[:])
        uaug_sb = consts.tile([DQK, 2, DQK + 1], F16)
        nc.scalar.dma_start(out=uaug_sb, in_=uaug2[:])
        bt_sb = consts.tile([POS, 2 * L], F16)
        nc.scalar.dma_start(out=bt_sb, in_=bt_const[:])
        wv_sb = consts.tile([128, 6, 128], F16)
        nc.scalar.dma_start(out=wv_sb, in_=wv2[:].rearrange("(c p) m -> p c m", p=128))

        # ---- long-lived per-head tensors ----
        qT2 = perb.tile([DQK + 1, 2, L], F16, name="qT2")
        kT2 = perb.tile([DQK + 1, 2, L], F16, name="kT2")
        qw2 = perb.tile([POS, 2, L], F16, name="qw2")
        v2 = perb.tile([128, NT, 2, DV + 1], F16, name="v2")
        nc.gpsimd.memset(qT2[DQK : DQK + 1, :, :], 1.0)
        nc.gpsimd.memset(v2[:, :, :, DV : DV + 1], 1.0)

        def proj_head(h):
            for ch in range(2):
                cols = slice(ch * 512, (ch + 1) * 512)
                ps_qk = ps_st.tile([128, 512], F32, tag="bank", name=f"ps_qk{h}{ch}")
                for ck in range(6):
                    nc.tensor.matmul(
                        ps_qk,
                        lhsT=wqk_sb[:, ck, h * 128 : (h + 1) * 128],
                        rhs=xT_sb[:, ck, cols],
                        start=(ck == 0),
                        stop=(ck == 5),
                    )
                nc.vector.tensor_copy(qT2[0:DQK, h, cols], ps_qk[0:DQK, :])
                nc.scalar.copy(kT2[0:DQK, h, cols], ps_qk[DQK:128, :])
            for ch in range(2):
                cols = slice(ch * 512, (ch + 1) * 512)
                ps_uk = ps_st.tile([DQK + 1, 512], F32, tag="bank", name=f"ps_uk{h}{ch}")
                nc.tensor.matmul(
                    ps_uk,
                    lhsT=uaug_sb[:, h, :],
                    rhs=kT2[0:DQK, h, cols],
                    start=True,
                    stop=True,
                )
                nc.vector.tensor_copy(
                    kT2[DQK : DQK + 1, h, cols], ps_uk[DQK : DQK + 1, :]
                )
                ps_qw = ps_st.tile([POS, 512], F32, tag="bank", name=f"ps_qw{h}{ch}")
                nc.tensor.matmul(
                    ps_qw,
                    lhsT=wposa_sb[:, h, :],
                    rhs=qT2[:, h, cols],
                    start=True,
                    stop=True,
                )
                nc.vector.tensor_copy(qw2[:, h, cols], ps_qw)

        def proj_v(jts):
            for jt in jts:
                ps_v = ps_st.tile([128, 128], F32, tag="bank", name=f"ps_v{jt}")
                for ck in range(6):
                    nc.tensor.matmul(
                        ps_v,
                        lhsT=xT_sb[:, ck, jt * 128 : (jt + 1) * 128],
                        rhs=wv_sb[:, ck, :],
                        start=(ck == 0),
                        stop=(ck == 5),
                    )
                nc.vector.tensor_copy(
                    v2[:, jt, :, 0:DV],
                    ps_v[:].rearrange("p (h d) -> p h d", h=2),
                )

        # AllToAll staging: a2a_in[t] = av pair tile [128 i, 2x64 hd] for
        # dest t.  A single post-loop collective: an AllToAll fired mid-loop
        # monopolizes the DMA engines and starves the pipeline (measured).
        a2a_in = dram.tile([N_CORES, 128, 128], F16, tag="a2a_in")
        a2a_out = dram.tile([N_CORES, 128, 128], F16, tag="a2a_out")

        # Basis sparsity: the widest center mask is (L+1)/2, so basis[d]=0
        # for |d|>512 -> strip cols outside [511-s_t, 1535-s_t] are zero;
        # and the skewed read never touches local col 1151.  Band per t
        # (always <= 1024 wide -> exactly 2 matmul chunks), stored in
        # STATIC per-t DRAM strips whose zero region is written once.
        band = []
        for t in range(NT):
            s_t = 896 - 128 * t
            lo = max(511 - s_t, 0)
            hi = min(1535 - s_t, 1150)
            band.append((lo, hi - lo + 1))
        strips8 = dram.tile([NT, 128, STRIP_W], F16, tag="strips8", name="strips8")
        zrow = consts.tile([128, STRIP_W], F16)
        nc.gpsimd.memset(zrow, 0.0)
        for t in range(NT):
            lo, w = band[t]
            if lo > 0:
                nc.scalar.dma_start(out=strips8[t, :, 0:lo], in_=zrow[:, 0:lo])
            if lo + w <= STRIP_W - 2:
                nc.scalar.dma_start(
                    out=strips8[t, :, lo + w : STRIP_W],
                    in_=zrow[:, lo + w : STRIP_W],
                )

        st_stage = {}
        st_strip = {}
        st_qr = {}
        st_ps = {}
        st_p = {}
        av_t = {}

        def unit(u):
            return u % 2, u // 2

        def StMM(v):
            h, t = unit(v)
            s_t = 896 - 128 * t
            stage = work.tile([128, STRIP_W], F16, tag="stage", name=f"stage{v}")
            lo, w = band[t]
            for c0, cw in ((lo, 512), (lo + 512, w - 512)):
                ps = ps_st.tile([128, cw], F32, tag="bank", name=f"ps_st{v}_{c0}")
                nc.tensor.matmul(
                    ps,
                    lhsT=qw2[:, h, t * 128 : (t + 1) * 128],
                    rhs=bt_sb[:, s_t + c0 : s_t + c0 + cw],
                    start=True,
                    stop=True,
                )
                nc.vector.tensor_copy(stage[:, c0 : c0 + cw], ps)
            st_stage[v] = stage

        def StOUT(v):
            h, t = unit(v)
            lo, w = band[t]
            stage = st_stage.pop(v)
            nc.scalar.dma_start(
                out=strips8[t, :, lo : lo + w], in_=stage[:, lo : lo + w]
            )

        def StSKEW(v):
            h, t = unit(v)
            qr_sb = work.tile([128, L], F16, tag="qr", name=f"qr{v}")
            src = bass.AP(
                tensor=strips8.tensor,
                offset=strips8.offset + t * 128 * STRIP_W + 127,
                ap=[[STRIP_W - 1, 128], [1, L]],
            )
            nc.sync.dma_start(out=qr_sb, in_=src)
            st_qr[v] = qr_sb

        def Sc(u):
            h, t = unit(u)
            qr_sb = st_qr.pop(u)
            ps_s = ps_sc.tile([128, L], F32, tag="scores", name=f"ps_s{u}")
            for ch in range(2):
                cols = slice(ch * 512, (ch + 1) * 512)
                nc.tensor.matmul(
                    ps_s[:, cols],
                    lhsT=qT2[:, h, t * 128 : (t + 1) * 128],
                    rhs=kT2[:, h, cols],
                    start=True,
                    stop=True,
                )
            # qr added by the DVE instead of PE identity matmuls:
            # ps_s += qr (fp16 SBUF read, fp32 PSUM accumulate).  Two
            # halves: half 0 runs while the PE still streams chunk 1, so
            # exp only waits the second (short) half.
            for ch in range(2):
                cols = slice(ch * 512, (ch + 1) * 512)
                nc.vector.scalar_tensor_tensor(
                    out=ps_s[:, cols],
                    in0=ps_s[:, cols],
                    scalar=1.0,
                    in1=qr_sb[:, cols],
                    op0=mybir.AluOpType.mult,
                    op1=mybir.AluOpType.add,
                )
            st_ps[u] = ps_s

        def BsH(u):
            ps_s = st_ps.pop(u)
            p_sb = pwork.tile([128, L], F16, tag="p", name=f"p{u}")
            pT_sb = pwork.tile([128, NT, 128], F16, tag="pT", name=f"pT{u}")
            nc.scalar.activation(
                out=p_sb,
                in_=ps_s,
                func=mybir.ActivationFunctionType.Exp,
                scale=float(scale),
            )
            nc.sync.dma_start_transpose(out=pT_sb, in_=p_sb)
            st_p[u] = pT_sb

        def Cav(u):
            h, t = unit(u)
            pT_sb = st_p.pop(u)
            ps_o = ps_av.tile([128, DV + 1], F32, tag="av", name=f"ps_o{u}")
            for jt in range(NT):
                nc.tensor.matmul(
                    ps_o,
                    lhsT=pT_sb[:, jt, :],
                    rhs=v2[:, jt, h, :],
                    start=(jt == 0),
                    stop=(jt == NT - 1),
                )
            if h == 0:
                av_t[t] = pwork.tile([128, 2, DV], F16, tag="av2", name=f"av2_{t}")
            rden = pwork.tile([128, 1], F32, tag="rden", name=f"rden{u}")
            nc.vector.reciprocal(rden, ps_o[:, DV : DV + 1])
            nc.vector.tensor_scalar_mul(av_t[t][:, h, :], in0=ps_o[:, 0:DV], scalar1=rden)

        def Cout(u):
            h, t = unit(u)
            if h != 1:
                return
            av_sb = av_t.pop(t)
            nc.scalar.dma_start(
                out=a2a_in[t, :, :], in_=av_sb[:].rearrange("p h d -> p (h d)")
            )

        # ---- seed: projections then strips ahead.  Each St substage is
        # emitted an iteration after its producer so every DMA issue's
        # dependencies are already complete (no live waits on the ACT
        # queue ahead of exp). ----
        # critical chain to Sc(0): proj_head(0) -> StMM/StOUT(0) -> skew(0);
        # everything else (h1 projections, v, strips 1-2) fills the gaps and
        # the v projection spreads over the first two loop iterations
        proj_head(0)
        StMM(0)
        StOUT(0)
        proj_head(1)
        StSKEW(0)
        StMM(1)
        StOUT(1)
        StMM(2)
        StOUT(2)
        StSKEW(1)

        # ---- main loop ----
        # StMM before Sc: the strip copies then precede the qr-add on the
        # DVE queue, so the PE's strip chunk 3 never waits a copy that is
        # itself queued behind the (live) qr-add.
        for u in range(NU):
            if u >= 2:
                Cav(u - 2)
            if u + 3 < NU:
                StMM(u + 3)
            Sc(u)
            if u < 2:
                proj_v(range(4 * u, 4 * u + 4))
            if u >= 1:
                BsH(u - 1)
            if u >= 2:
                Cout(u - 2)
            # StSKEW before StOUT: units (h0,t) and (h1,t) share strips8[t],
            # so h0's skewed read must be ordered before h1's band write
            if u + 2 < NU:
                StSKEW(u + 2)
            if u + 3 < NU:
                StOUT(u + 3)
        BsH(NU - 1)
        Cav(NU - 2)
        Cout(NU - 2)
        Cav(NU - 1)
        Cout(NU - 1)

        # tail-only weights: issued late so startup DMA goes to x/wqk
        wo_sb = consts.tile([128, 4, DM], F16)
        nc.scalar.dma_start(out=wo_sb, in_=wo[:].rearrange("(c p) m -> p c m", p=128))
        bo_sb = consts.tile([1, DM], F16)
        nc.scalar.dma_start(out=bo_sb, in_=bo[:])
        ones_sb = consts.tile([1, 128], F16)
        nc.vector.memset(ones_sb, 1.0)

        # ---- AllToAll reshard: i-tiles -> cores ----
        nc.gpsimd.collective_compute(
            "AllToAll",
            mybir.AluOpType.bypass,
            replica_groups=[list(range(N_CORES))],
            ins=[a2a_in.opt()],
            outs=[a2a_out.opt()],
        )
        # unpack: avall[p, it, a*128+d2] = a2a_out[it*4+a, p, d2]
        # (it = batch of my shard half; a = head-pair rank within batch)
        avall = consts.tile([128, 2, 512], F16)
        for it in range(2):
            src = bass.AP(
                tensor=a2a_out.tensor,
                offset=a2a_out.offset + it * 4 * 128 * 128,
                ap=[[128, 128], [128 * 128, 4], [1, 128]],
            )
            nc.sync.dma_start(out=avall[:, it, :], in_=src)
        # hd-major for the projection: 2 batched XBAR transposes (sync only)
        avT_all = consts.tile([128, 4, SHARD], F16)
        for it in range(2):
            nc.sync.dma_start_transpose(
                out=avT_all[:, :, it * 128 : (it + 1) * 128],
                in_=avall[:, it, :],
            )

        # ---- output projection on own 256 rows: [256, 512] @ [512, 768] + bo
        for it in range(SHARD // 128):
            ps_proj = ps_sc.tile([128, DM], F32, tag="scores", name=f"ps_proj{it}")
            for cols in (slice(0, 512), slice(512, DM)):
                for cc in range(4):
                    nc.tensor.matmul(
                        ps_proj[:, cols],
                        lhsT=avT_all[:, cc, it * 128 : (it + 1) * 128],
                        rhs=wo_sb[:, cc, cols],
                        start=(cc == 0),
                        stop=False,
                    )
                nc.tensor.matmul(
                    ps_proj[:, cols],
                    lhsT=ones_sb,
                    rhs=bo_sb[:, cols],
                    start=False,
                    stop=True,
                )
            o_sb = work.tile([128, DM], F32, tag="osb", name=f"osb{it}")
            if it == 0:
                nc.vector.tensor_copy(o_sb, ps_proj)
            else:
                nc.scalar.copy(o_sb, ps_proj)
            nc.sync.dma_start(out=out[it * 128 : (it + 1) * 128, :], in_=o_sb)


_PROGRAM = None


def _get_program():
    global _PROGRAM
    if _PROGRAM is None:
        _PROGRAM = _build_program()
    return _PROGRAM


def _in_maps(x, Wq, Wk, Wv, Wo, bo, u_bias, v_bias, w_pos):
    f16 = np.float16
    woT = np.ascontiguousarray(Wo.T).astype(f16)
    bo_row = np.ascontiguousarray(bo[None, :]).astype(f16)
    xT_b = [
        np.ascontiguousarray(x[b].T).astype(f16) for b in range(B)
    ]  # [768, 1024] each
    maps = []
    for c in range(N_CORES):
        b, a = divmod(c, 4)
        h0, h1 = 2 * a, 2 * a + 1
        wqk_parts = []
        wposa_h, uaug_h = [], []
        for h in (h0, h1):
            sl = slice(h * DQK, (h + 1) * DQK)
            wqk_parts.append(np.concatenate([Wq[sl].T, Wk[sl].T], axis=1))
            wposa_h.append(
                np.concatenate([w_pos[h], (w_pos[h].T @ v_bias[h])[None, :]], axis=0)
            )
            ua = np.zeros((DQK, DQK + 1), f16)
            ua[:, DQK] = u_bias[h].astype(f16)
            uaug_h.append(ua)
        wv2 = np.concatenate(
            [Wv[h0 * DV : (h0 + 1) * DV].T, Wv[h1 * DV : (h1 + 1) * DV].T], axis=1
        )
        # pre-transpose to load layout: wposa2 [65, 2, 64], uaug2 [64, 2, 65]
        wposa2 = np.ascontiguousarray(
            np.stack(wposa_h).transpose(1, 0, 2)
        ).astype(f16)
        uaug2 = np.ascontiguousarray(np.stack(uaug_h).transpose(1, 0, 2))
        maps.append(
            {
                "xT": xT_b[b],
                "wqk2": np.ascontiguousarray(
                    np.concatenate(wqk_parts, axis=1)
                ).astype(f16),
                "wv2": np.ascontiguousarray(wv2).astype(f16),
                "wposa2": wposa2,
                "uaug2": uaug2,
                "wo": woT,
                "bo": bo_row,
            }
        )
    return maps


def kernel(x, Wq, Wk, Wv, Wo, bo, u_bias, v_bias, w_pos, _trace=False):
    nc = _get_program()
    maps = _in_maps(
        np.asarray(x), np.asarray(Wq), np.asarray(Wk), np.asarray(Wv),
        np.asarray(Wo), np.asarray(bo), np.asarray(u_bias), np.asarray(v_bias),
        np.asarray(w_pos),
    )
    res = run_bass_kernel_spmd(
        nc, maps, core_ids=list(range(N_CORES)), trace=_trace
    )
    # core c returns [256, 768]: rows 0:128 = batch0 rows [c*128, +128),
    # rows 128:256 = batch1 rows [c*128, +128).
    full = np.empty((B, L, DM), np.float32)
    for c in range(N_CORES):
        shard = res.results[c]["out_shard"]
        full[0, c * 128 : (c + 1) * 128] = shard[0:128]
        full[1, c * 128 : (c + 1) * 128] = shard[128:256]
    if _trace:
        kernel.last_exec_time_ns = res.exec_time_ns
        kernel.last_results = res
    return full


if __name__ == "__main__":
    # quick self-check against the reference on CPU
    import jax

    sys.path.insert(0, "/root/problem")
    cpu = jax.devices("cpu")[0]
    import reference

    with jax.default_device(cpu):
        inputs = reference.setup_inputs()
        expected = np.asarray(reference.reference(**inputs))
    got = kernel(**{k: np.asarray(v) for k, v in inputs.items()})
    rel = np.abs(got - expected).max() / np.abs(expected).max()
    print(f"Relative error: {rel:.3e}")


# revision 76
# speedup vs baseline: 1.1002x; 1.1002x over previous
"""Enformer multi-head attention with central-mask relative position bias.

Trainium2 Bass/Tile kernel, sharded over 8 NeuronCores.

Problem (fp32): x [2, 1024, 768]; H=8 heads, dqk=dv=64, n_pos=64.
  q,k,v = x @ {Wq,Wk,Wv}.T ; basis[i,j,:] = f(j-i)  (Toeplitz!)
  qr = (q @ w_pos) . basis ; uk = u.k ; vr = (v_bias.w_pos) . basis
  scores = (q.k + qr + uk + vr)/8 ; out = softmax(scores) @ v @ Wo.T + bo

Sharding: core c owns heads {2a, 2a+1} (a = c%4) of batch c//4, so
each core touches only one batch's x (1.5 MB) and the v projection
streams 128 output columns (2 heads at once).  The output is
row-sharded so dest core c owns i-tile t=c of BOTH batches: every src
has exactly one [128 i, 128 hd] av pair tile per dest and the reshard
is a uniform 8-way AllToAll (256 KB, post-loop; firing a collective
mid-loop starves the pipeline because it monopolizes the DMA engines).

Pipeline (16 units u: h=u%2, t=u//2).  Stages are staggered so a DMA
issue never head-of-line blocks a dependent queue behind a live wait
(in particular, exp is always first on the ACT queue each iteration):
  StMM(v)  @ iter v-3: strip matmuls + PSUM->SBUF copies (DVE)
  StOUT(v) @ iter v-3: strip SBUF->DRAM bounce          (scalar queue)
  StSKEW(v)@ iter v-2: skewed DRAM->SBUF read, 1 DMA     (scalar queue)
The v projection is spread over the first two loop iterations so the
first scores matmul starts as soon as head 0's projections land.
  Sc(u)    @ iter u:   scores q.k (2 matmuls); qr then added into the
             scores PSUM by DVE scalar_tensor_tensor (no PE identity
             matmuls)
  BsH(u)   @ iter u+1: exp [128,1024] (ACT) + p XBAR transpose (sync)
  Cav(u)   @ iter u+2: attn@v (65 cols: col 64 = den via ones column
             in v) + recip + scale (DVE)
  Cout(u)  @ iter u+2, h=1 only: av pair DMA to the AllToAll staging
             buffer (scalar queue)
All XBAR transposes stay on the sync queue: two concurrent
InstDmaTransposeAnt on different queues corrupt each other (observed
on HW: only row 0 of each 16-row xbar tile lands).

Relative-position trick: basis[i,j,:] = B[j-i+1023, :] depends only on
the diagonal, so qr[i,j] + vr[i,j] = T'[i, j-i+1023] with T' = (qw+vw)
@ B.T.  T' is computed in 128-row strips [128, 1152] (the diagonal
window of one i-tile), bounced through DRAM, and read back with a
skewed access pattern (partition stride 1151 elements) that turns
diagonals into rows.  uk[j] is folded into the scores matmul as a 65th
contraction row (q row 64 = ones, k row 64 = uk); vw is folded into the
qw matmul the same way.
"""

import sys

sys.path.insert(0, "/opt/trn_rl_repo")

import numpy as np

import concourse.bass as bass
import concourse.mybir as mybir
import concourse.tile as tile
from concourse import bacc
from concourse.bass_utils import run_bass_kernel_spmd

N_CORES = 8
B, L, DM = 2, 1024, 768
H, DQK, DV, POS = 8, 64, 64, 64
ROWS = B * L             # 2048
SHARD = ROWS // N_CORES  # 256
NT = L // 128            # 8 i-tiles per batch
NU = 16                  # units per core: 2 heads x 8 i-tiles
STRIP_W = 1152           # 3 matmul chunks: 512 + 512 + 128 (window is 1151)
F32 = mybir.dt.float32
F16 = mybir.dt.float16


def _basis_bt() -> np.ndarray:
    """B.T [64, 2048] fp16: basis value for signed distance d = r - 1023.

    Values are in {-1, 0, 1}: exact in fp16.  Col 2047 is padding.
    """
    half = POS // 2
    d = np.arange(-(L - 1), L, dtype=np.int64)  # [2047]
    log_v = np.log(np.float32((L + 1) / 2.0)).astype(np.float32)
    pow_rate = np.exp(log_v / np.float32(half)).astype(np.float32)
    widths = (pow_rate ** np.arange(1, half + 1, dtype=np.float32)).astype(np.float32)
    unsigned = np.abs(d)[:, None].astype(np.float32) <= widths[None, :]
    signed = np.sign(d)[:, None] * unsigned
    bmat = np.concatenate(
        [unsigned.astype(np.float32), signed.astype(np.float32)], axis=1
    )  # [2047, 64]
    bt = np.zeros((POS, 2 * L), np.float32)
    bt[:, : 2 * L - 1] = bmat.T
    return bt.astype(np.float16)


def _build_program():
    nc = bacc.Bacc("TRN2", target_bir_lowering=False, debug=False, num_devices=N_CORES)

    xT = nc.dram_tensor("xT", [DM, L], F16, kind="ExternalInput")
    wqk2 = nc.dram_tensor("wqk2", [DM, 256], F16, kind="ExternalInput")
    wv2 = nc.dram_tensor("wv2", [DM, 128], F16, kind="ExternalInput")
    # pre-transposed on host: [65, 2, 64] / [64, 2, 65] load contiguously
    wposa2 = nc.dram_tensor("wposa2", [DQK + 1, 2, POS], F16, kind="ExternalInput")
    uaug2 = nc.dram_tensor("uaug2", [DQK, 2, DQK + 1], F16, kind="ExternalInput")
    wo = nc.dram_tensor("wo", [H * DV, DM], F16, kind="ExternalInput")
    bo = nc.dram_tensor("bo", [1, DM], F16, kind="ExternalInput")
    out = nc.dram_tensor("out_shard", [SHARD, DM], F32, kind="ExternalOutput")

    bt_const = nc.inline_tensor(_basis_bt(), name="bt_const")

    with tile.TileContext(nc) as tc:
        _emit(nc, tc, xT, wqk2, wv2, wposa2, uaug2, wo, bo, bt_const, out)
    nc.compile()
    return nc


def _emit(nc, tc, xT, wqk2, wv2, wposa2, uaug2, wo, bo, bt_const, out):
    import contextlib

    ctx = contextlib.ExitStack()
    with ctx:
        consts = ctx.enter_context(tc.tile_pool(name="consts", bufs=1))
        perb = ctx.enter_context(tc.tile_pool(name="perb", bufs=1))
        work = ctx.enter_context(tc.tile_pool(name="work", bufs=6))
        pwork = ctx.enter_context(tc.tile_pool(name="pwork", bufs=5))
        ps_st = ctx.enter_context(tc.tile_pool(name="ps_st", bufs=2, space="PSUM"))
        ps_av = ctx.enter_context(tc.tile_pool(name="ps_av", bufs=2, space="PSUM"))
        ps_sc = ctx.enter_context(tc.tile_pool(name="ps_sc", bufs=2, space="PSUM"))
        dram = ctx.enter_context(tc.tile_pool(name="dram", bufs=6, space="DRAM"))

        scale = 1.0 / np.sqrt(DQK)

        # ---- constants: x on sync, weights on scalar (parallel queues) ----
        xT_sb = consts.tile([128, 6, L], F16)
        for cc in range(2):
            nc.sync.dma_start(
                out=xT_sb[:, :, cc * 512 : (cc + 1) * 512],
                in_=xT[:].rearrange("(c p) i -> p c i", p=128)[
                    :, :, cc * 512 : (cc + 1) * 512
                ],
            )
        wqk_sb = consts.tile([128, 6, 256], F16)
        nc.scalar.dma_start(out=wqk_sb, in_=wqk2[:].rearrange("(c p) m -> p c m", p=128))
        wposa_sb = consts.tile([DQK + 1, 2, POS], F16)
        nc.scalar.dma_start(out=wposa_sb, in_=wposa2[:])
        uaug_sb = consts.tile([DQK, 2, DQK + 1], F16)
        nc.scalar.dma_start(out=uaug_sb, in_=uaug2[:])
        bt_sb = consts.tile([POS, 2 * L], F16)
        nc.scalar.dma_start(out=bt_sb, in_=bt_const[:])
        wv_sb = consts.tile([128, 6, 128], F16)
        nc.scalar.dma_start(out=wv_sb, in_=wv2[:].rearrange("(c p) m -> p c m", p=128))

        # ---- long-lived per-head tensors ----
        qT2 = perb.tile([DQK + 1, 2, L], F16, name="qT2")
        kT2 = perb.tile([DQK + 1, 2, L], F16, name="kT2")
        qw2 = perb.tile([POS, 2, L], F16, name="qw2")
        v2 = perb.tile([128, NT, 2, DV + 1], F16, name="v2")
        nc.gpsimd.memset(qT2[DQK : DQK + 1, :, :], 1.0)
        nc.gpsimd.memset(v2[:, :, :, DV : DV + 1], 1.0)

        def proj_head(h):
            for ch in range(2):
                cols = slice(ch * 512, (ch + 1) * 512)
                ps_qk = ps_st.tile([128, 512], F32, tag="bank", name=f"ps_qk{h}{ch}")
                for ck in range(6):
                    nc.tensor.matmul(
                        ps_qk,
                        lhsT=wqk_sb[:, ck, h * 128 : (h + 1) * 128],
                        rhs=xT_sb[:, ck, cols],
                        start=(ck == 0),
                        stop=(ck == 5),
                    )
                nc.vector.tensor_copy(qT2[0:DQK, h, cols], ps_qk[0:DQK, :])
                nc.scalar.copy(kT2[0:DQK, h, cols], ps_qk[DQK:128, :])
            for ch in range(2):
                cols = slice(ch * 512, (ch + 1) * 512)
                ps_uk = ps_st.tile([DQK + 1, 512], F32, tag="bank", name=f"ps_uk{h}{ch}")
                nc.tensor.matmul(
                    ps_uk,
                    lhsT=uaug_sb[:, h, :],
                    rhs=kT2[0:DQK, h, cols],
                    start=True,
                    stop=True,
                )
                nc.vector.tensor_copy(
                    kT2[DQK : DQK + 1, h, cols], ps_uk[DQK : DQK + 1, :]
                )
                ps_qw = ps_st.tile([POS, 512], F32, tag="bank", name=f"ps_qw{h}{ch}")
                nc.tensor.matmul(
                    ps_qw,
                    lhsT=wposa_sb[:, h, :],
                    rhs=qT2[:, h, cols],
                    start=True,
                    stop=True,
                )
                nc.vector.tensor_copy(qw2[:, h, cols], ps_qw)

        def proj_v(jts):
            for jt in jts:
                ps_v = ps_st.tile([128, 128], F32, tag="bank", name=f"ps_v{jt}")
                for ck in range(6):
                    nc.tensor.matmul(
                        ps_v,
                        lhsT=xT_sb[:, ck, jt * 128 : (jt + 1) * 128],
                        rhs=wv_sb[:, ck, :],
                        start=(ck == 0),
                        stop=(ck == 5),
                    )
                nc.vector.tensor_copy(
                    v2[:, jt, :, 0:DV],
                    ps_v[:].rearrange("p (h d) -> p h d", h=2),
                )

        # AllToAll staging: a2a_in[t] = av pair tile [128 i, 2x64 hd] for
        # dest t.  A single post-loop collective: an AllToAll fired mid-loop
        # monopolizes the DMA engines and starves the pipeline (measured).
        a2a_in = dram.tile([N_CORES, 128, 128], F16, tag="a2a_in")
        a2a_out = dram.tile([N_CORES, 128, 128], F16, tag="a2a_out")

        # Basis sparsity: the widest center mask is (L+1)/2, so basis[d]=0
        # for |d|>512 -> strip cols outside [511-s_t, 1535-s_t] are zero;
        # and the skewed read never touches local col 1151.  Band per t
        # (always <= 1024 wide -> exactly 2 matmul chunks), stored in
        # STATIC per-t DRAM strips whose zero region is written once.
        band = []
        for t in range(NT):
            s_t = 896 - 128 * t
            lo = max(511 - s_t, 0)
            hi = min(1535 - s_t, 1150)
            band.append((lo, hi - lo + 1))
        strips8 = dram.tile([NT, 128, STRIP_W], F16, tag="strips8", name="strips8")
        zrow = consts.tile([128, STRIP_W], F16)
        nc.gpsimd.memset(zrow, 0.0)
        for t in range(NT):
            lo, w = band[t]
            if lo > 0:
                nc.sync.dma_start(out=strips8[t, :, 0:lo], in_=zrow[:, 0:lo])
            if lo + w <= STRIP_W - 2:
                nc.sync.dma_start(
                    out=strips8[t, :, lo + w : STRIP_W],
                    in_=zrow[:, lo + w : STRIP_W],
                )

        st_stage = {}
        st_strip = {}
        st_qr = {}
        st_ps = {}
        st_p = {}
        av_t = {}

        def unit(u):
            return u % 2, u // 2

        def StMM(v):
            h, t = unit(v)
            s_t = 896 - 128 * t
            stage = work.tile([128, STRIP_W], F16, tag="stage", name=f"stage{v}")
            lo, w = band[t]
            for c0, cw in ((lo, 512), (lo + 512, w - 512)):
                ps = ps_st.tile([128, cw], F32, tag="bank", name=f"ps_st{v}_{c0}")
                nc.tensor.matmul(
                    ps,
                    lhsT=qw2[:, h, t * 128 : (t + 1) * 128],
                    rhs=bt_sb[:, s_t + c0 : s_t + c0 + cw],
                    start=True,
                    stop=True,
                )
                nc.vector.tensor_copy(stage[:, c0 : c0 + cw], ps)
            st_stage[v] = stage

        def StOUT(v):
            h, t = unit(v)
            lo, w = band[t]
            stage = st_stage.pop(v)
            nc.scalar.dma_start(
                out=strips8[t, :, lo : lo + w], in_=stage[:, lo : lo + w]
            )

        def StSKEW(v):
            h, t = unit(v)
            qr_sb = work.tile([128, L], F16, tag="qr", name=f"qr{v}")
            src = bass.AP(
                tensor=strips8.tensor,
                offset=strips8.offset + t * 128 * STRIP_W + 127,
                ap=[[STRIP_W - 1, 128], [1, L]],
            )
            nc.sync.dma_start(out=qr_sb, in_=src)
            st_qr[v] = qr_sb

        def Sc(u):
            h, t = unit(u)
            qr_sb = st_qr.pop(u)
            ps_s = ps_sc.tile([128, L], F32, tag="scores", name=f"ps_s{u}")
            for ch in range(2):
                cols = slice(ch * 512, (ch + 1) * 512)
                nc.tensor.matmul(
                    ps_s[:, cols],
                    lhsT=qT2[:, h, t * 128 : (t + 1) * 128],
                    rhs=kT2[:, h, cols],
                    start=True,
                    stop=True,
                )
            # qr added by the DVE instead of PE identity matmuls:
            # ps_s += qr (fp16 SBUF read, fp32 PSUM accumulate).  Two
            # halves: half 0 runs while the PE still streams chunk 1, so
            # exp only waits the second (short) half.
            for ch in range(2):
                cols = slice(ch * 512, (ch + 1) * 512)
                nc.vector.scalar_tensor_tensor(
                    out=ps_s[:, cols],
                    in0=ps_s[:, cols],
                    scalar=1.0,
                    in1=qr_sb[:, cols],
                    op0=mybir.AluOpType.mult,
                    op1=mybir.AluOpType.add,
                )
            st_ps[u] = ps_s

        def BsH(u):
            ps_s = st_ps.pop(u)
            p_sb = pwork.tile([128, L], F16, tag="p", name=f"p{u}")
            pT_sb = pwork.tile([128, NT, 128], F16, tag="pT", name=f"pT{u}")
            nc.scalar.activation(
                out=p_sb,
                in_=ps_s,
                func=mybir.ActivationFunctionType.Exp,
                scale=float(scale),
            )
            nc.sync.dma_start_transpose(out=pT_sb, in_=p_sb)
            st_p[u] = pT_sb

        def Cav(u):
            h, t = unit(u)
            pT_sb = st_p.pop(u)
            ps_o = ps_av.tile([128, DV + 1], F32, tag="av", name=f"ps_o{u}")
            for jt in range(NT):
                nc.tensor.matmul(
                    ps_o,
                    lhsT=pT_sb[:, jt, :],
                    rhs=v2[:, jt, h, :],
                    start=(jt == 0),
                    stop=(jt == NT - 1),
                )
            if h == 0:
                av_t[t] = pwork.tile([128, 2, DV], F16, tag="av2", name=f"av2_{t}")
            rden = pwork.tile([128, 1], F32, tag="rden", name=f"rden{u}")
            nc.vector.reciprocal(rden, ps_o[:, DV : DV + 1])
            nc.vector.tensor_scalar_mul(av_t[t][:, h, :], in0=ps_o[:, 0:DV], scalar1=rden)

        def Cout(u):
            h, t = unit(u)
            if h != 1:
                return
            av_sb = av_t.pop(t)
            nc.scalar.dma_start(
                out=a2a_in[t, :, :], in_=av_sb[:].rearrange("p h d -> p (h d)")
            )

        # ---- seed: projections then strips ahead.  Each St substage is
        # emitted an iteration after its producer so every DMA issue's
        # dependencies are already complete (no live waits on the ACT
        # queue ahead of exp). ----
        # critical chain to Sc(0): proj_head(0) -> StMM/StOUT(0) -> skew(0);
        # everything else (h1 projections, v, strips 1-2) fills the gaps and
        # the v projection spreads over the first two loop iterations
        proj_head(0)
        StMM(0)
        StOUT(0)
        proj_head(1)
        StSKEW(0)
        StMM(1)
        StOUT(1)
        StMM(2)
        StOUT(2)
        StSKEW(1)
        StSKEW(2)
        StMM(3)
        StOUT(3)

        # ---- main loop ----
        # StMM before Sc: the strip copies then precede the qr-add on the
        # DVE queue, so the PE's strip chunk 3 never waits a copy that is
        # itself queued behind the (live) qr-add.
        for u in range(NU):
            if u >= 2:
                Cav(u - 2)
            if u + 4 < NU:
                StMM(u + 4)
            Sc(u)
            if u < 2:
                proj_v(range(4 * u, 4 * u + 4))
            if u >= 1:
                BsH(u - 1)
            if u >= 2:
                Cout(u - 2)
            # StSKEW before StOUT: units (h0,t) and (h1,t) share strips8[t],
            # so h0's skewed read must be ordered before h1's band write
            if u + 3 < NU:
                StSKEW(u + 3)
            if u + 4 < NU:
                StOUT(u + 4)
        BsH(NU - 1)
        Cav(NU - 2)
        Cout(NU - 2)
        Cav(NU - 1)
        Cout(NU - 1)

        # tail-only weights: issued late so startup DMA goes to x/wqk
        wo_sb = consts.tile([128, 4, DM], F16)
        nc.scalar.dma_start(out=wo_sb, in_=wo[:].rearrange("(c p) m -> p c m", p=128))
        bo_sb = consts.tile([1, DM], F16)
        nc.scalar.dma_start(out=bo_sb, in_=bo[:])
        ones_sb = consts.tile([1, 128], F16)
        nc.vector.memset(ones_sb, 1.0)

        # ---- AllToAll reshard: i-tiles -> cores ----
        nc.gpsimd.collective_compute(
            "AllToAll",
            mybir.AluOpType.bypass,
            replica_groups=[list(range(N_CORES))],
            ins=[a2a_in.opt()],
            outs=[a2a_out.opt()],
        )
        # unpack: avall[p, it, a*128+d2] = a2a_out[it*4+a, p, d2]
        # (it = batch of my shard half; a = head-pair rank within batch)
        avall = consts.tile([128, 2, 512], F16)
        for it in range(2):
            src = bass.AP(
                tensor=a2a_out.tensor,
                offset=a2a_out.offset + it * 4 * 128 * 128,
                ap=[[128, 128], [128 * 128, 4], [1, 128]],
            )
            nc.sync.dma_start(out=avall[:, it, :], in_=src)
        # hd-major for the projection: 2 batched XBAR transposes (sync only)
        avT_all = consts.tile([128, 4, SHARD], F16)
        for it in range(2):
            nc.sync.dma_start_transpose(
                out=avT_all[:, :, it * 128 : (it + 1) * 128],
                in_=avall[:, it, :],
            )

        # ---- output projection on own 256 rows: [256, 512] @ [512, 768] + bo
        for it in range(SHARD // 128):
            ps_proj = ps_sc.tile([128, DM], F32, tag="scores", name=f"ps_proj{it}")
            for cols in (slice(0, 512), slice(512, DM)):
                for cc in range(4):
                    nc.tensor.matmul(
                        ps_proj[:, cols],
                        lhsT=avT_all[:, cc, it * 128 : (it + 1) * 128],
                        rhs=wo_sb[:, cc, cols],
                        start=(cc == 0),
                        stop=False,
                    )
                nc.tensor.matmul(
                    ps_proj[:, cols],
                    lhsT=ones_sb,
                    rhs=bo_sb[:, cols],
                    start=False,
                    stop=True,
                )
            o_sb = work.tile([128, DM], F32, tag="osb", name=f"osb{it}")
            if it == 0:
                nc.vector.tensor_copy(o_sb, ps_proj)
            else:
                nc.scalar.copy(o_sb, ps_proj)
            nc.sync.dma_start(out=out[it * 128 : (it + 1) * 128, :], in_=o_sb)


_PROGRAM = None


def _get_program():
    global _PROGRAM
    if _PROGRAM is None:
        _PROGRAM = _build_program()
    return _PROGRAM


def _in_maps(x, Wq, Wk, Wv, Wo, bo, u_bias, v_bias, w_pos):
    f16 = np.float16
    woT = np.ascontiguousarray(Wo.T).astype(f16)
    bo_row = np.ascontiguousarray(bo[None, :]).astype(f16)
    xT_b = [
        np.ascontiguousarray(x[b].T).astype(f16) for b in range(B)
    ]  # [768, 1024] each
    maps = []
    for c in range(N_CORES):
        b, a = divmod(c, 4)
        h0, h1 = 2 * a, 2 * a + 1
        wqk_parts = []
        wposa_h, uaug_h = [], []
        for h in (h0, h1):
            sl = slice(h * DQK, (h + 1) * DQK)
            wqk_parts.append(np.concatenate([Wq[sl].T, Wk[sl].T], axis=1))
            wposa_h.append(
                np.concatenate([w_pos[h], (w_pos[h].T @ v_bias[h])[None, :]], axis=0)
            )
            ua = np.zeros((DQK, DQK + 1), f16)
            ua[:, DQK] = u_bias[h].astype(f16)
            uaug_h.append(ua)
        wv2 = np.concatenate(
            [Wv[h0 * DV : (h0 + 1) * DV].T, Wv[h1 * DV : (h1 + 1) * DV].T], axis=1
        )
        # pre-transpose to load layout: wposa2 [65, 2, 64], uaug2 [64, 2, 65]
        wposa2 = np.ascontiguousarray(
            np.stack(wposa_h).transpose(1, 0, 2)
        ).astype(f16)
        uaug2 = np.ascontiguousarray(np.stack(uaug_h).transpose(1, 0, 2))
        maps.append(
            {
                "xT": xT_b[b],
                "wqk2": np.ascontiguousarray(
                    np.concatenate(wqk_parts, axis=1)
                ).astype(f16),
                "wv2": np.ascontiguousarray(wv2).astype(f16),
                "wposa2": wposa2,
                "uaug2": uaug2,
                "wo": woT,
                "bo": bo_row,
            }
        )
    return maps


def kernel(x, Wq, Wk, Wv, Wo, bo, u_bias, v_bias, w_pos, _trace=False):
    nc = _get_program()
    maps = _in_maps(
        np.asarray(x), np.asarray(Wq), np.asarray(Wk), np.asarray(Wv),
        np.asarray(Wo), np.asarray(bo), np.asarray(u_bias), np.asarray(v_bias),
        np.asarray(w_pos),
    )
    res = run_bass_kernel_spmd(
        nc, maps, core_ids=list(range(N_CORES)), trace=_trace
    )
    # core c returns [256, 768]: rows 0:128 = batch0 rows [c*128, +128),
    # rows 128:256 = batch1 rows [c*128, +128).
    full = np.empty((B, L, DM), np.float32)
    for c in range(N_CORES):
        shard = res.results[c]["out_shard"]
        full[0, c * 128 : (c + 1) * 128] = shard[0:128]
        full[1, c * 128 : (c + 1) * 128] = shard[128:256]
    if _trace:
        kernel.last_exec_time_ns = res.exec_time_ns
        kernel.last_results = res
    return full


if __name__ == "__main__":
    # quick self-check against the reference on CPU
    import jax

    sys.path.insert(0, "/root/problem")
    cpu = jax.devices("cpu")[0]
    import reference

    with jax.default_device(cpu):
        inputs = reference.setup_inputs()
        expected = np.asarray(reference.reference(**inputs))
    got = kernel(**{k: np.asarray(v) for k, v in inputs.items()})
    rel = np.abs(got - expected).max() / np.abs(expected).max()
    print(f"Relative error: {rel:.3e}")


# revision 80
# speedup vs baseline: 1.1734x; 1.0665x over previous
"""Enformer multi-head attention with central-mask relative position bias.

Trainium2 Bass/Tile kernel, sharded over 8 NeuronCores.

Problem (fp32): x [2, 1024, 768]; H=8 heads, dqk=dv=64, n_pos=64.
  q,k,v = x @ {Wq,Wk,Wv}.T ; basis[i,j,:] = f(j-i)  (Toeplitz!)
  qr = (q @ w_pos) . basis ; uk = u.k ; vr = (v_bias.w_pos) . basis
  scores = (q.k + qr + uk + vr)/8 ; out = softmax(scores) @ v @ Wo.T + bo

Sharding: core c owns heads {2a, 2a+1} (a = c%4) of batch c//4, so
each core touches only one batch's x (1.5 MB) and the v projection
streams 128 output columns (2 heads at once).  The output is
row-sharded so dest core c owns i-tile t=c of BOTH batches: every src
has exactly one [128 i, 128 hd] av pair tile per dest and the reshard
is a uniform 8-way AllToAll (256 KB, post-loop; firing a collective
mid-loop starves the pipeline because it monopolizes the DMA engines).

Pipeline (16 units u: h=u%2, t=u//2).  Stages are staggered so a DMA
issue never head-of-line blocks a dependent queue behind a live wait
(in particular, exp is always first on the ACT queue each iteration):
  StMM(v)  @ iter v-3: strip matmuls + PSUM->SBUF copies (DVE)
  StOUT(v) @ iter v-3: strip SBUF->DRAM bounce          (scalar queue)
  StSKEW(v)@ iter v-2: skewed DRAM->SBUF read, 1 DMA     (scalar queue)
The v projection is spread over the first two loop iterations so the
first scores matmul starts as soon as head 0's projections land.
  Sc(u)    @ iter u:   scores q.k (2 matmuls); qr then added into the
             scores PSUM by DVE scalar_tensor_tensor (no PE identity
             matmuls)
  BsH(u)   @ iter u+1: exp [128,1024] (ACT) + p XBAR transpose (sync)
  Cav(u)   @ iter u+2: attn@v (65 cols: col 64 = den via ones column
             in v) + recip + scale (DVE)
  Cout(u)  @ iter u+2, h=1 only: av pair DMA to the AllToAll staging
             buffer (scalar queue)
All XBAR transposes stay on the sync queue: two concurrent
InstDmaTransposeAnt on different queues corrupt each other (observed
on HW: only row 0 of each 16-row xbar tile lands).

Relative-position trick: basis[i,j,:] = B[j-i+1023, :] depends only on
the diagonal, so qr[i,j] + vr[i,j] = T'[i, j-i+1023] with T' = (qw+vw)
@ B.T.  T' is computed in 128-row strips [128, 1152] (the diagonal
window of one i-tile), bounced through DRAM, and read back with a
skewed access pattern (partition stride 1151 elements) that turns
diagonals into rows.  uk[j] is folded into the scores matmul as a 65th
contraction row (q row 64 = ones, k row 64 = uk); vw is folded into the
qw matmul the same way.
"""

import sys

sys.path.insert(0, "/opt/trn_rl_repo")

import numpy as np

import concourse.bass as bass
import concourse.mybir as mybir
import concourse.tile as tile
from concourse import bacc
from concourse.bass_utils import run_bass_kernel_spmd

N_CORES = 8
B, L, DM = 2, 1024, 768
H, DQK, DV, POS = 8, 64, 64, 64
ROWS = B * L             # 2048
SHARD = ROWS // N_CORES  # 256
NT = L // 128            # 8 i-tiles per batch
NU = 16                  # units per core: 2 heads x 8 i-tiles
STRIP_W = 1152           # 3 matmul chunks: 512 + 512 + 128 (window is 1151)
F32 = mybir.dt.float32
F16 = mybir.dt.float16


def _basis_bt() -> np.ndarray:
    """B.T [64, 2048] fp16: basis value for signed distance d = r - 1023.

    Values are in {-1, 0, 1}: exact in fp16.  Col 2047 is padding.
    """
    half = POS // 2
    d = np.arange(-(L - 1), L, dtype=np.int64)  # [2047]
    log_v = np.log(np.float32((L + 1) / 2.0)).astype(np.float32)
    pow_rate = np.exp(log_v / np.float32(half)).astype(np.float32)
    widths = (pow_rate ** np.arange(1, half + 1, dtype=np.float32)).astype(np.float32)
    unsigned = np.abs(d)[:, None].astype(np.float32) <= widths[None, :]
    signed = np.sign(d)[:, None] * unsigned
    bmat = np.concatenate(
        [unsigned.astype(np.float32), signed.astype(np.float32)], axis=1
    )  # [2047, 64]
    bt = np.zeros((POS, 2 * L), np.float32)
    bt[:, : 2 * L - 1] = bmat.T
    return bt.astype(np.float16)


def _build_program():
    nc = bacc.Bacc("TRN2", target_bir_lowering=False, debug=False, num_devices=N_CORES)

    xT = nc.dram_tensor("xT", [DM, L], F16, kind="ExternalInput")
    wqk2 = nc.dram_tensor("wqk2", [DM, 256], F16, kind="ExternalInput")
    wv2 = nc.dram_tensor("wv2", [DM, 128], F16, kind="ExternalInput")
    # pre-transposed on host: [65, 2, 64] / [64, 2, 65] load contiguously
    wposa2 = nc.dram_tensor("wposa2", [DQK + 1, 2, POS], F16, kind="ExternalInput")
    uaug2 = nc.dram_tensor("uaug2", [DQK, 2, DQK + 1], F16, kind="ExternalInput")
    wo = nc.dram_tensor("wo", [H * DV, DM], F16, kind="ExternalInput")
    bo = nc.dram_tensor("bo", [1, DM], F16, kind="ExternalInput")
    out = nc.dram_tensor("out_shard", [SHARD, DM], F32, kind="ExternalOutput")

    bt_const = nc.inline_tensor(_basis_bt(), name="bt_const")

    with tile.TileContext(nc) as tc:
        _emit(nc, tc, xT, wqk2, wv2, wposa2, uaug2, wo, bo, bt_const, out)
    nc.compile()
    return nc


def _emit(nc, tc, xT, wqk2, wv2, wposa2, uaug2, wo, bo, bt_const, out):
    import contextlib

    ctx = contextlib.ExitStack()
    with ctx:
        consts = ctx.enter_context(tc.tile_pool(name="consts", bufs=1))
        perb = ctx.enter_context(tc.tile_pool(name="perb", bufs=1))
        work = ctx.enter_context(tc.tile_pool(name="work", bufs=6))
        pwork = ctx.enter_context(tc.tile_pool(name="pwork", bufs=5))
        ps_st = ctx.enter_context(tc.tile_pool(name="ps_st", bufs=2, space="PSUM"))
        ps_av = ctx.enter_context(tc.tile_pool(name="ps_av", bufs=2, space="PSUM"))
        ps_sc = ctx.enter_context(tc.tile_pool(name="ps_sc", bufs=2, space="PSUM"))
        dram = ctx.enter_context(tc.tile_pool(name="dram", bufs=6, space="DRAM"))

        scale = 1.0 / np.sqrt(DQK)

        # ---- constants: x on sync, weights on scalar (parallel queues) ----
        xT_sb = consts.tile([128, 6, L], F16)
        for cc in range(2):
            nc.sync.dma_start(
                out=xT_sb[:, :, cc * 512 : (cc + 1) * 512],
                in_=xT[:].rearrange("(c p) i -> p c i", p=128)[
                    :, :, cc * 512 : (cc + 1) * 512
                ],
            )
        wqk_sb = consts.tile([128, 6, 256], F16)
        nc.scalar.dma_start(out=wqk_sb, in_=wqk2[:].rearrange("(c p) m -> p c m", p=128))
        wposa_sb = consts.tile([DQK + 1, 2, POS], F16)
        nc.scalar.dma_start(out=wposa_sb, in_=wposa2[:])
        uaug_sb = consts.tile([DQK, 2, DQK + 1], F16)
        nc.scalar.dma_start(out=uaug_sb, in_=uaug2[:])
        bt_sb = consts.tile([POS, 2 * L], F16)
        nc.scalar.dma_start(out=bt_sb, in_=bt_const[:])
        wv_sb = consts.tile([128, 6, 128], F16)
        nc.scalar.dma_start(out=wv_sb, in_=wv2[:].rearrange("(c p) m -> p c m", p=128))

        # ---- long-lived per-head tensors ----
        qT2 = perb.tile([DQK + 1, 2, L], F16, name="qT2")
        kT2 = perb.tile([DQK + 1, 2, L], F16, name="kT2")
        qw2 = perb.tile([POS, 2, L], F16, name="qw2")
        v2 = perb.tile([128, NT, 2, DV + 1], F16, name="v2")
        nc.gpsimd.memset(qT2[DQK : DQK + 1, :, :], 1.0)
        nc.gpsimd.memset(v2[:, :, :, DV : DV + 1], 1.0)

        def proj_head(h):
            for ch in range(2):
                cols = slice(ch * 512, (ch + 1) * 512)
                ps_qk = ps_st.tile([128, 512], F32, tag="bank", name=f"ps_qk{h}{ch}")
                for ck in range(6):
                    nc.tensor.matmul(
                        ps_qk,
                        lhsT=wqk_sb[:, ck, h * 128 : (h + 1) * 128],
                        rhs=xT_sb[:, ck, cols],
                        start=(ck == 0),
                        stop=(ck == 5),
                    )
                nc.vector.tensor_copy(qT2[0:DQK, h, cols], ps_qk[0:DQK, :])
                nc.scalar.copy(kT2[0:DQK, h, cols], ps_qk[DQK:128, :])
            for ch in range(2):
                cols = slice(ch * 512, (ch + 1) * 512)
                ps_uk = ps_st.tile([DQK + 1, 512], F32, tag="bank", name=f"ps_uk{h}{ch}")
                nc.tensor.matmul(
                    ps_uk,
                    lhsT=uaug_sb[:, h, :],
                    rhs=kT2[0:DQK, h, cols],
                    start=True,
                    stop=True,
                )
                nc.vector.tensor_copy(
                    kT2[DQK : DQK + 1, h, cols], ps_uk[DQK : DQK + 1, :]
                )
                ps_qw = ps_st.tile([POS, 512], F32, tag="bank", name=f"ps_qw{h}{ch}")
                nc.tensor.matmul(
                    ps_qw,
                    lhsT=wposa_sb[:, h, :],
                    rhs=qT2[:, h, cols],
                    start=True,
                    stop=True,
                )
                nc.vector.tensor_copy(qw2[:, h, cols], ps_qw)

        def proj_v(jts):
            for jt in jts:
                ps_v = ps_st.tile([128, 128], F32, tag="bank", name=f"ps_v{jt}")
                for ck in range(6):
                    nc.tensor.matmul(
                        ps_v,
                        lhsT=xT_sb[:, ck, jt * 128 : (jt + 1) * 128],
                        rhs=wv_sb[:, ck, :],
                        start=(ck == 0),
                        stop=(ck == 5),
                    )
                nc.vector.tensor_copy(
                    v2[:, jt, :, 0:DV],
                    ps_v[:].rearrange("p (h d) -> p h d", h=2),
                )

        # AllToAll staging: a2a_in[t] = av pair tile [128 i, 2x64 hd] for
        # dest t.  A single post-loop collective: an AllToAll fired mid-loop
        # monopolizes the DMA engines and starves the pipeline (measured).
        a2a_in = dram.tile([N_CORES, 128, 128], F16, tag="a2a_in")
        a2a_out = dram.tile([N_CORES, 128, 128], F16, tag="a2a_out")

        # Basis sparsity: the widest center mask is (L+1)/2, so basis[d]=0
        # for |d|>512 -> strip cols outside [511-s_t, 1535-s_t] are zero;
        # and the skewed read never touches local col 1151.  Band per t
        # (always <= 1024 wide -> exactly 2 matmul chunks), stored in
        # STATIC per-t DRAM strips whose zero region is written once.
        band = []
        for t in range(NT):
            s_t = 896 - 128 * t
            lo = max(511 - s_t, 0)
            hi = min(1535 - s_t, 1150)
            band.append((lo, hi - lo + 1))
        strips8 = dram.tile([NT, 128, STRIP_W], F16, tag="strips8", name="strips8")
        zrow = consts.tile([128, STRIP_W], F16)
        nc.gpsimd.memset(zrow, 0.0)
        for t in range(NT):
            lo, w = band[t]
            if lo > 0:
                nc.sync.dma_start(out=strips8[t, :, 0:lo], in_=zrow[:, 0:lo])
            if lo + w <= STRIP_W - 2:
                nc.sync.dma_start(
                    out=strips8[t, :, lo + w : STRIP_W],
                    in_=zrow[:, lo + w : STRIP_W],
                )

        st_stage = {}
        st_strip = {}
        st_qr = {}
        st_ps = {}
        st_p = {}
        av_t = {}

        def unit(u):
            return u % 2, u // 2

        def StMM(v):
            h, t = unit(v)
            s_t = 896 - 128 * t
            stage = work.tile([128, STRIP_W], F16, tag="stage", name=f"stage{v}")
            lo, w = band[t]
            for c0, cw in ((lo, 512), (lo + 512, w - 512)):
                ps = ps_st.tile([128, cw], F32, tag="bank", name=f"ps_st{v}_{c0}")
                nc.tensor.matmul(
                    ps,
                    lhsT=qw2[:, h, t * 128 : (t + 1) * 128],
                    rhs=bt_sb[:, s_t + c0 : s_t + c0 + cw],
                    start=True,
                    stop=True,
                )
                nc.vector.tensor_copy(stage[:, c0 : c0 + cw], ps)
            st_stage[v] = stage

        def StOUT(v):
            h, t = unit(v)
            lo, w = band[t]
            stage = st_stage.pop(v)
            nc.scalar.dma_start(
                out=strips8[t, :, lo : lo + w], in_=stage[:, lo : lo + w]
            )

        def StSKEW(v):
            h, t = unit(v)
            qr_sb = work.tile([128, L], F16, tag="qr", name=f"qr{v}")
            src = bass.AP(
                tensor=strips8.tensor,
                offset=strips8.offset + t * 128 * STRIP_W + 127,
                ap=[[STRIP_W - 1, 128], [1, L]],
            )
            nc.sync.dma_start(out=qr_sb, in_=src)
            st_qr[v] = qr_sb

        def Sc(u):
            h, t = unit(u)
            qr_sb = st_qr.pop(u)
            ps_s = ps_sc.tile([128, L], F32, tag="scores", name=f"ps_s{u}")
            for ch in range(2):
                cols = slice(ch * 512, (ch + 1) * 512)
                nc.tensor.matmul(
                    ps_s[:, cols],
                    lhsT=qT2[:, h, t * 128 : (t + 1) * 128],
                    rhs=kT2[:, h, cols],
                    start=True,
                    stop=True,
                )
            # qr added by the DVE instead of PE identity matmuls:
            # ps_s += qr (fp16 SBUF read, fp32 PSUM accumulate).  Two
            # halves: half 0 runs while the PE still streams chunk 1, so
            # exp only waits the second (short) half.
            for ch in range(2):
                cols = slice(ch * 512, (ch + 1) * 512)
                nc.vector.scalar_tensor_tensor(
                    out=ps_s[:, cols],
                    in0=ps_s[:, cols],
                    scalar=1.0,
                    in1=qr_sb[:, cols],
                    op0=mybir.AluOpType.mult,
                    op1=mybir.AluOpType.add,
                )
            st_ps[u] = ps_s

        def BsH(u):
            ps_s = st_ps.pop(u)
            p_sb = pwork.tile([128, L], F16, tag="p", name=f"p{u}")
            pT_sb = pwork.tile([128, NT, 128], F16, tag="pT", name=f"pT{u}")
            nc.scalar.activation(
                out=p_sb,
                in_=ps_s,
                func=mybir.ActivationFunctionType.Exp,
                scale=float(scale),
            )
            nc.sync.dma_start_transpose(out=pT_sb, in_=p_sb)
            st_p[u] = pT_sb

        def Cav(u):
            h, t = unit(u)
            pT_sb = st_p.pop(u)
            ps_o = ps_av.tile([128, DV + 1], F32, tag="av", name=f"ps_o{u}")
            for jt in range(NT):
                nc.tensor.matmul(
                    ps_o,
                    lhsT=pT_sb[:, jt, :],
                    rhs=v2[:, jt, h, :],
                    start=(jt == 0),
                    stop=(jt == NT - 1),
                )
            if h == 0:
                av_t[t] = pwork.tile([128, 2, DV], F16, tag="av2", name=f"av2_{t}")
            rden = pwork.tile([128, 1], F32, tag="rden", name=f"rden{u}")
            nc.vector.reciprocal(rden, ps_o[:, DV : DV + 1])
            nc.vector.tensor_scalar_mul(av_t[t][:, h, :], in0=ps_o[:, 0:DV], scalar1=rden)

        def Cout(u):
            h, t = unit(u)
            if h != 1:
                return
            av_sb = av_t.pop(t)
            nc.scalar.dma_start(
                out=a2a_in[t, :, :], in_=av_sb[:].rearrange("p h d -> p (h d)")
            )

        # ---- seed: projections then strips ahead.  Each St substage is
        # emitted an iteration after its producer so every DMA issue's
        # dependencies are already complete (no live waits on the ACT
        # queue ahead of exp). ----
        # critical chain to Sc(0): proj_head(0) -> StMM/StOUT(0) -> skew(0);
        # everything else (h1 projections, v, strips 1-2) fills the gaps and
        # the v projection spreads over the first two loop iterations
        proj_head(0)
        StMM(0)
        StOUT(0)
        proj_head(1)
        StSKEW(0)
        StMM(1)
        StOUT(1)
        StMM(2)
        StOUT(2)
        StSKEW(1)
        StSKEW(2)
        StMM(3)
        StOUT(3)

        # ---- main loop ----
        # StMM before Sc: the strip copies then precede the qr-add on the
        # DVE queue, so the PE's strip chunk 3 never waits a copy that is
        # itself queued behind the (live) qr-add.
        for u in range(NU):
            if u >= 2:
                Cav(u - 2)
            if u + 4 < NU:
                StMM(u + 4)
            Sc(u)
            if u < 2:
                proj_v(range(4 * u, 4 * u + 4))
            if u >= 1:
                BsH(u - 1)
            if u >= 2:
                Cout(u - 2)
            # StSKEW before StOUT: units (h0,t) and (h1,t) share strips8[t],
            # so h0's skewed read must be ordered before h1's band write
            if u + 3 < NU:
                StSKEW(u + 3)
            if u + 4 < NU:
                StOUT(u + 4)
        BsH(NU - 1)
        Cav(NU - 2)
        Cout(NU - 2)
        Cav(NU - 1)
        Cout(NU - 1)

        # tail-only weights: issued late so startup DMA goes to x/wqk
        wo_sb = consts.tile([128, 4, DM], F16)
        nc.scalar.dma_start(out=wo_sb, in_=wo[:].rearrange("(c p) m -> p c m", p=128))
        bo_sb = consts.tile([1, DM], F16)
        nc.scalar.dma_start(out=bo_sb, in_=bo[:])
        ones_sb = consts.tile([1, 128], F16)
        nc.vector.memset(ones_sb, 1.0)

        # ---- AllToAll reshard: i-tiles -> cores ----
        nc.gpsimd.collective_compute(
            "AllToAll",
            mybir.AluOpType.bypass,
            replica_groups=[list(range(N_CORES))],
            ins=[a2a_in.opt()],
            outs=[a2a_out.opt()],
        )
        # unpack: avall[p, it, a*128+d2] = a2a_out[it*4+a, p, d2]
        # (it = batch of my shard half; a = head-pair rank within batch)
        avall = consts.tile([128, 2, 512], F16)
        for it in range(2):
            src = bass.AP(
                tensor=a2a_out.tensor,
                offset=a2a_out.offset + it * 4 * 128 * 128,
                ap=[[128, 128], [128 * 128, 4], [1, 128]],
            )
            nc.sync.dma_start(out=avall[:, it, :], in_=src)
        # hd-major for the projection: 2 batched XBAR transposes (sync only)
        avT_all = consts.tile([128, 4, SHARD], F16)
        for it in range(2):
            nc.sync.dma_start_transpose(
                out=avT_all[:, :, it * 128 : (it + 1) * 128],
                in_=avall[:, it, :],
            )

        # ---- output projection on own 256 rows: [256, 512] @ [512, 768] + bo
        for it in range(SHARD // 128):
            ps_proj = ps_sc.tile([128, DM], F32, tag="scores", name=f"ps_proj{it}")
            for cols in (slice(0, 512), slice(512, DM)):
                for cc in range(4):
                    nc.tensor.matmul(
                        ps_proj[:, cols],
                        lhsT=avT_all[:, cc, it * 128 : (it + 1) * 128],
                        rhs=wo_sb[:, cc, cols],
                        start=(cc == 0),
                        stop=False,
                    )
                nc.tensor.matmul(
                    ps_proj[:, cols],
                    lhsT=ones_sb,
                    rhs=bo_sb[:, cols],
                    start=False,
                    stop=True,
                )
            o_sb = work.tile([128, DM], F32, tag="osb", name=f"osb{it}")
            if it == 0:
                nc.vector.tensor_copy(o_sb, ps_proj)
            else:
                nc.scalar.copy(o_sb, ps_proj)
            nc.sync.dma_start(out=out[it * 128 : (it + 1) * 128, :], in_=o_sb)


_PROGRAM = None


def _get_program():
    global _PROGRAM
    if _PROGRAM is None:
        _PROGRAM = _build_program()
    return _PROGRAM


def _in_maps(x, Wq, Wk, Wv, Wo, bo, u_bias, v_bias, w_pos):
    f16 = np.float16
    woT = np.ascontiguousarray(Wo.T).astype(f16)
    bo_row = np.ascontiguousarray(bo[None, :]).astype(f16)
    xT_b = [
        np.ascontiguousarray(x[b].T).astype(f16) for b in range(B)
    ]  # [768, 1024] each
    maps = []
    for c in range(N_CORES):
        b, a = divmod(c, 4)
        h0, h1 = 2 * a, 2 * a + 1
        wqk_parts = []
        wposa_h, uaug_h = [], []
        for h in (h0, h1):
            sl = slice(h * DQK, (h + 1) * DQK)
            wqk_parts.append(np.concatenate([Wq[sl].T, Wk[sl].T], axis=1))
            wposa_h.append(
                np.concatenate([w_pos[h], (w_pos[h].T @ v_bias[h])[None, :]], axis=0)
            )
            ua = np.zeros((DQK, DQK + 1), f16)
            ua[:, DQK] = u_bias[h].astype(f16)
            uaug_h.append(ua)
        wv2 = np.concatenate(
            [Wv[h0 * DV : (h0 + 1) * DV].T, Wv[h1 * DV : (h1 + 1) * DV].T], axis=1
        )
        # pre-transpose to load layout: wposa2 [65, 2, 64], uaug2 [64, 2, 65]
        wposa2 = np.ascontiguousarray(
            np.stack(wposa_h).transpose(1, 0, 2)
        ).astype(f16)
        uaug2 = np.ascontiguousarray(np.stack(uaug_h).transpose(1, 0, 2))
        maps.append(
            {
                "xT": xT_b[b],
                "wqk2": np.ascontiguousarray(
                    np.concatenate(wqk_parts, axis=1)
                ).astype(f16),
                "wv2": np.ascontiguousarray(wv2).astype(f16),
                "wposa2": wposa2,
                "uaug2": uaug2,
                "wo": woT,
                "bo": bo_row,
            }
        )
    return maps


def kernel(x, Wq, Wk, Wv, Wo, bo, u_bias, v_bias, w_pos, _trace=False):
    nc = _get_program()
    maps = _in_maps(
        np.asarray(x), np.asarray(Wq), np.asarray(Wk), np.asarray(Wv),
        np.asarray(Wo), np.asarray(bo), np.asarray(u_bias), np.asarray(v_bias),
        np.asarray(w_pos),
    )
    res = run_bass_kernel_spmd(
        nc, maps, core_ids=list(range(N_CORES)), trace=_trace
    )
    # core c returns [256, 768]: rows 0:128 = batch0 rows [c*128, +128),
    # rows 128:256 = batch1 rows [c*128, +128).
    full = np.empty((B, L, DM), np.float32)
    for c in range(N_CORES):
        shard = res.results[c]["out_shard"]
        full[0, c * 128 : (c + 1) * 128] = shard[0:128]
        full[1, c * 128 : (c + 1) * 128] = shard[128:256]
    if _trace:
        kernel.last_exec_time_ns = res.exec_time_ns
        kernel.last_results = res
    return full


if __name__ == "__main__":
    # quick self-check against the reference on CPU
    import jax

    sys.path.insert(0, "/root/problem")
    cpu = jax.devices("cpu")[0]
    import reference

    with jax.default_device(cpu):
        inputs = reference.setup_inputs()
        expected = np.asarray(reference.reference(**inputs))
    got = kernel(**{k: np.asarray(v) for k, v in inputs.items()})
    rel = np.abs(got - expected).max() / np.abs(expected).max()
    print(f"Relative error: {rel:.3e}")
